# revision 22
# baseline (speedup 1.0000x reference)
"""Single-head self-attention over 8192 assets (D=512) on 8 TRN2 NeuronCores.

Sequence-parallel over rows: core i owns queries [i*1024, (i+1)*1024).
k/v for the core's own tokens are projected locally (bf16 matmuls, fp32
PSUM), cast to fp8e4 (RTN, validated bit-exact vs numpy on HW), and
shared via 4 split fp8 AllGathers (halved bytes vs bf16; 2MB-out
gathers run ~105GB/s vs ~70 for 1MB) that pipeline against the
attention sweep.  Each core ALSO projects its right
neighbor's 1024 tokens locally (slots 8-15 of the rotated key layout
are exactly rank i+1), which fills the CC stream's ~21+40us entry
barrier + first-gather window with useful PE work; the gathers then
only feed ranks i+2..i+7 (6 chunks per slice = 3 aligned pairs).

Math/accuracy choices (numpy-validated, rel err ~1.45e-2 vs 2e-2 gate;
HW matches the numpy model to ~1e-4 because every fp8 cast is RTN):
  - bk is dropped entirely: q.(k+bk) adds a per-row constant to scores,
    which softmax cancels exactly.
  - bv is added after normalization (softmax rows sum to 1) as a
    fused DVE (hs x rcp) + bv_rep op, deleting 16 PE seed matmuls from
    the projection critical path and keeping bv out of fp8 entirely.
  - scores use a PARTIAL q-SPLIT in fp8 DoubleRow: q ~ q_hi + q_lo
    (both e4m3; the residual exercises fp8 subnormals, exact on HW),
    with the lo-correction applied to 3/8 of the key chunks.  Plain-fp8 q
    fails (2.1e-2: its quantization tilts whole softmax rows
    coherently); correcting a 3/8 fraction scales the tilt to 1.62e-2
    while saving one 263ns PE instruction slot per skipped pass -- the
    PE is dispatch-bound at ~263ns per matmul regardless of size.
  - attn @ v is plain fp8 DoubleRow on pT=exp(scores) and v: the one
    true fp8 2x win (errors here average over 8192 keys).
  - denominators: ones-stationary DoubleRow matmuls over cached pT,
    deferred until after the sweep so PSUM stays within 8 banks.

The PE issues one matmul per ~263ns independent of operand width, so
the design minimizes instruction count; matmul outputs are hard-capped
at one 2KB PSUM bank (walrus rejects bank-spanning outs).

PSUM schedule (8 banks): hs0 [128,4,512] parked for the whole mb0 sweep
(4) + two rotating score pair tiles [128,2,512] (2+2).  mb1's attention
is deferred: its exp(scores) land in a 64KB/partition SBUF cache during
the sweep, and its attention+den run as a pure-PE phase afterwards in
the banks the score tiles vacate.  exp is one ACT op per score pair,
reading PSUM as a flat [128,1024] AP (measured 2x faster than two
[128,512] ops)."""

import numpy as np
import ml_dtypes

import concourse.mybir as mybir
from concourse.bass import _add_dep_helper as bass_dep, ds as bass_ds
import concourse.tile as tile
from concourse import bacc
from concourse.bass_utils import run_bass_kernel_spmd

N_CORES = 8
N_TOK = 8192
D = 512
M_LOC = N_TOK // N_CORES   # 1024 query rows per core / tokens per kv shard
P = 128                    # SBUF partitions
DC = D // P                # 4 chunks of the latent dim
MB = 2                     # query blocks of 512
NSL = 4                    # split gathers (slices)
STOK = M_LOC // NSL        # tokens per rank per slice (256)
NCH = N_TOK // P           # 64 key chunks
NPR = NCH // 2             # 32 key-chunk pairs
SCALE = float(1.0 / np.sqrt(D))

F32 = mybir.dt.float32
BF16 = mybir.dt.bfloat16
F8 = mybir.dt.float8e4

KT_SL = D * STOK           # kT slice elems per rank (65536)
V_SL = STOK * D            # v slice elems per rank (65536)
KV_SL = KT_SL + V_SL


def _build():
    nc = bacc.Bacc("TRN2", target_bir_lowering=False, debug=False,
                   num_devices=N_CORES)

    zT_d = nc.dram_tensor("zT_loc", [D, 2 * M_LOC], BF16, kind="ExternalInput")
    WqT_d = nc.dram_tensor("WqT", [D, D], BF16, kind="ExternalInput")
    WkT_d = nc.dram_tensor("WkT", [D, D], BF16, kind="ExternalInput")
    WvT_d = nc.dram_tensor("WvT", [D, D], BF16, kind="ExternalInput")
    bq_d = nc.dram_tensor("bq", [D], F32, kind="ExternalInput")
    bv_rep_d = nc.dram_tensor("bv_rep", [P, D], F32, kind="ExternalInput")
    ones8_d = nc.dram_tensor("ones8", [P, 2, P], F8, kind="ExternalInput")
    offs_d = nc.dram_tensor("offs", [1, 2 * (N_CORES - 2)], mybir.dt.int32,
                            kind="ExternalInput")

    h_d = nc.dram_tensor("h_out", [M_LOC, D], F32, kind="ExternalOutput")

    kv_in = [nc.dram_tensor(f"kv_in{a}", [KV_SL], F8) for a in range(NSL)]
    kv_all = [nc.dram_tensor(f"kv_all{a}", [N_CORES * KV_SL], F8,
                             addr_space="Shared") for a in range(NSL)]

    def kt_view(flat):
        return flat[0:KT_SL].rearrange("(p c m) -> p c m", p=P, c=DC)

    def v_view(flat):
        return flat[KT_SL:KV_SL].rearrange("(p t d) -> p t d", p=P, t=2)

    with tile.TileContext(nc) as tc:
        with (
            tc.tile_pool(name="const", bufs=1) as const,
            tc.tile_pool(name="persist", bufs=1) as persist,
        ):
            # ---- constants / weights (k-proj needs come first) ----
            from contextlib import ExitStack
            proj_ctx = ExitStack()
            proj = proj_ctx.enter_context(tc.tile_pool(name="proj", bufs=1))
            ps_proj = proj_ctx.enter_context(
                tc.tile_pool(name="ps_proj", bufs=6, space="PSUM"))

            zT_sb = proj.tile([P, DC, 2 * M_LOC], BF16)
            zT_dv = zT_d.ap().rearrange("(c p) m -> p c m", p=P)
            WqT_sb = proj.tile([P, DC, D], BF16)
            WkT_sb = proj.tile([P, DC, D], BF16)
            WvT_sb = proj.tile([P, DC, D], BF16)
            q_bf = proj.tile([P, DC, M_LOC], BF16)
            bq_sb = const.tile([P, DC], F32)
            bv_rep = const.tile([P, D], F32)
            ones8 = const.tile([P, 2, P], F8)
            zeros_col = const.tile([P, 1], F32)
            offs_sb = persist.tile([1, 2 * (N_CORES - 2)], mybir.dt.int32)

            # first k-proj group needs WkT + zT[0:512]: split them across
            # the two HWDGE rings so both land as early as possible
            nc.sync.dma_start(WkT_sb[:], WkT_d.ap().rearrange("(c p) d -> p c d", p=P))
            nc.scalar.dma_start(zT_sb[:, :, 0:512], zT_dv[:, :, 0:512])
            nc.sync.dma_start(zT_sb[:, :, 512:M_LOC], zT_dv[:, :, 512:M_LOC])
            nc.sync.dma_start(zT_sb[:, :, M_LOC:2 * M_LOC],
                              zT_dv[:, :, M_LOC:2 * M_LOC])
            nc.scalar.dma_start(WvT_sb[:], WvT_d.ap().rearrange("(c p) d -> p c d", p=P))
            nc.scalar.dma_start(WqT_sb[:], WqT_d.ap().rearrange("(c p) d -> p c d", p=P))
            nc.scalar.dma_start(bq_sb[:], bq_d.ap().rearrange("(c p) -> p c", p=P))
            nc.scalar.dma_start(bv_rep[:], bv_rep_d[:])
            nc.scalar.dma_start(ones8[:], ones8_d[:])
            nc.scalar.dma_start(offs_sb[:], offs_d[:])
            nc.gpsimd.memset(zeros_col[:], 0.0)

            # ---- PE warm-up: the tensor engine clocks up only after ~3us
            # of sustained activity (0.65 -> 1.2 -> 2.4 GHz).  The first
            # ~12us are DMA-bound anyway, so burn them on garbage matmuls
            # over memset-zero tiles to hit full clock when real
            # projection work arrives.
            wu_st = const.tile([P, 2, P], F8)
            wu_mv = const.tile([P, 2, 512], F8)
            nc.gpsimd.memset(wu_st[:], 0.0)
            nc.gpsimd.memset(wu_mv[:], 0.0)
            for _ in range(40):
                ps = ps_proj.tile([P, 512], F32, name="ps")
                nc.tensor.matmul(ps[:], wu_st[:], wu_mv[:],
                                 start=True, stop=True,
                                 perf_mode=mybir.MatmulPerfMode.DoubleRow)

            # ---- persistent fp8 state ----
            # token layout: [own 1024][rank (i+1)%8: 1024]...[rank (i+7)%8]
            # (a per-core key permutation; softmax is order-invariant)
            kT8 = persist.tile([P, DC, N_TOK], F8)
            v8 = persist.tile([P, NCH, D], F8)
            qhi = persist.tile([P, DC, M_LOC], F8)
            qlo = persist.tile([P, DC, M_LOC], F8)
            pT_cache = persist.tile([P, MB, NPR, 2, D], F8)

            cc_insts = []

            def bounce(a):
                nc.sync.dma_start(kt_view(kv_in[a].ap()),
                                  kT8[:, :, a * STOK:(a + 1) * STOK])
                nc.sync.dma_start(v_view(kv_in[a].ap()),
                                  v8[:, 2 * a:2 * a + 2, :])
                cc = nc.gpsimd.collective_compute(
                    "AllGather",
                    mybir.AluOpType.bypass,
                    replica_groups=[list(range(N_CORES))],
                    ins=[kv_in[a].ap().opt()],
                    outs=[kv_all[a].ap().opt()],
                )
                cc_insts.append(cc)

            # ---- projections (bf16 matmuls, fp32 PSUM, fp8 casts) ----
            def k_proj(mh):
                for dc in range(DC):
                    ps = ps_proj.tile([P, 512], F32, name="ps")
                    for c in range(DC):
                        nc.tensor.matmul(
                            ps[:],
                            WkT_sb[:, c, dc * P:(dc + 1) * P],
                            zT_sb[:, c, mh * 512:(mh + 1) * 512],
                            start=(c == 0), stop=(c == DC - 1),
                        )
                    nc.scalar.copy(kT8[:, dc, mh * 512:(mh + 1) * 512], ps[:])

            def v_proj(t):
                ps = ps_proj.tile([P, 512], F32, name="ps")
                for c in range(DC):
                    nc.tensor.matmul(
                        ps[:],
                        zT_sb[:, c, t * P:(t + 1) * P],
                        WvT_sb[:, c, :],
                        start=(c == 0), stop=(c == DC - 1),
                    )
                nc.scalar.copy(v8[:, t, :], ps[:])

            # interleave so each gather fires as soon as its slice exists
            k_proj(0)
            for t in range(4):
                v_proj(t)
            bounce(0)
            bounce(1)
            k_proj(1)
            for t in range(4, 8):
                v_proj(t)
            bounce(2)
            bounce(3)
            # right-neighbor shard (slots 8..15 = rank i+1): projected
            # locally instead of waiting for its gathered copy, filling the
            # CC barrier window with useful PE work
            k_proj(2)
            k_proj(3)
            for t in range(8, 16):
                v_proj(t)

            # q projection + fp8 hi/lo split (subs split DVE/Pool)
            for mh in range(MB):
                for dc in range(DC):
                    ps = ps_proj.tile([P, 512], F32, name="ps")
                    for c in range(DC):
                        nc.tensor.matmul(
                            ps[:],
                            WqT_sb[:, c, dc * P:(dc + 1) * P],
                            zT_sb[:, c, mh * 512:(mh + 1) * 512],
                            start=(c == 0), stop=(c == DC - 1),
                        )
                    sl = (slice(None), dc, slice(mh * 512, (mh + 1) * 512))
                    nc.scalar.activation(
                        q_bf[sl], ps[:],
                        mybir.ActivationFunctionType.Identity,
                        bias=bq_sb[:, dc:dc + 1],
                    )
                    nc.scalar.copy(qhi[sl], q_bf[sl])
                    eng = nc.vector if dc % 2 == 0 else nc.gpsimd
                    eng.tensor_sub(qlo[sl], q_bf[sl], qhi[sl])

            proj_ctx.close()

            # ---- attention ----
            NR = N_CORES - 2
            kt_rv = [nc.values_load(offs_sb[0:1, j:j + 1],
                                    engines={mybir.EngineType.SP})
                     for j in range(NR)]
            v_rv = [nc.values_load(offs_sb[0:1, NR + j:NR + j + 1],
                                   engines={mybir.EngineType.SP})
                    for j in range(NR)]

            # remote kv slice reads (after each gather lands).  Destination
            # token slots follow PROCESSING order: own 0..7, then slice a
            # rank j -> slot 8 + 7a + j.  This is a per-core key
            # permutation (softmax is key-order invariant) that makes every
            # attention pair (2i, 2i+1) memory-consecutive in kT8/v8.
            for a in range(NSL):
                for j in range(NR):
                    t = 16 + a * 2 * NR + 2 * j
                    d1 = nc.sync.dma_start(
                        kT8[:, :, t * P:(t + 2) * P],
                        kv_all[a].ap()[bass_ds(kt_rv[j], KT_SL)]
                        .rearrange("(p c m) -> p c m", p=P, c=DC))
                    d2 = nc.sync.dma_start(
                        v8[:, t:t + 2, :],
                        kv_all[a].ap()[bass_ds(v_rv[j], V_SL)]
                        .rearrange("(p t d) -> p t d", p=P, t=2))
                    for dd in (d1, d2):
                        bass_dep(dd.ins, cc_insts[a].ins, sync=True,
                                 reason="dyn kv read after gather")

            def attn(hs, mb, pr, start, stop):
                for mt in range(4):
                    nc.tensor.matmul(
                        hs[:, mt, :],
                        pT_cache[:, mb, pr, :, mt * P:(mt + 1) * P],
                        v8[:, 2 * pr:2 * pr + 2, :],
                        start=start, stop=stop,
                        perf_mode=mybir.MatmulPerfMode.DoubleRow,
                    )

            def den_pass(ps_den, mb):
                for pr in range(NPR):
                    nc.tensor.matmul(
                        ps_den[:], ones8[:],
                        pT_cache[:, mb, pr],
                        start=(pr == 0), stop=(pr == NPR - 1),
                        perf_mode=mybir.MatmulPerfMode.DoubleRow,
                    )

            def norm_out(hs, ps_den, mb, scr, rcpw):
                h_dv = h_d.ap().rearrange("(t p) d -> p t d", p=P)
                for mt in range(4):
                    for x in range(4):
                        nc.vector.transpose(
                            scr[32 * x:32 * x + 32, mt * 32:(mt + 1) * 32],
                            ps_den[32 * x:32 * x + 32,
                                   mt * P + 32 * x:mt * P + 32 * x + 32])
                    nc.vector.reciprocal(rcpw[:, mt:mt + 1],
                                         scr[:, mt * 32:mt * 32 + 1])
                    ot = outp.tile([P, D], F32, name="ot", tag="ot")
                    nc.vector.scalar_tensor_tensor(
                        ot[:], hs[:, mt, :], rcpw[:, mt:mt + 1], bv_rep[:],
                        op0=mybir.AluOpType.mult,
                        op1=mybir.AluOpType.add)
                    nc.sync.dma_start(h_dv[:, mb * 4 + mt, :], ot[:])

            scr = persist.tile([P, 4 * 32], F32)
            rcpw = persist.tile([P, 4, 4], F32)
            outp_ctx = ExitStack()
            outp = outp_ctx.enter_context(tc.tile_pool(name="outp", bufs=2))

            with tc.tile_pool(name="ps_hs", bufs=1, space="PSUM") as ps_hs:
                hs0 = ps_hs.tile([P, 4, D], F32)

                sc_ctx = ExitStack()
                ps_sc = sc_ctx.enter_context(
                    tc.tile_pool(name="ps_sc", bufs=2, space="PSUM"))

                def scores(pr, mb):
                    sc = ps_sc.tile([P, 2, D], F32, name="sc", tag="sc")
                    qs = slice(mb * 512, (mb + 1) * 512)
                    # q_lo correction on slot-0 chunks only: the coherent
                    # q-quantization tilt scales with the uncorrected key
                    # fraction (numpy: half-split 1.46e-2 vs 2e-2 gate) and
                    # every skipped pass saves a 263ns PE instruction slot
                    for slot in range(2):
                        u = 2 * pr + slot
                        qvs = ((qhi, qlo) if slot == 0 and pr % 4 != 3
                               else (qhi,))
                        for cp in range(2):
                            kst = kT8[:, 2 * cp:2 * cp + 2, u * P:(u + 1) * P]
                            for i, qv in enumerate(qvs):
                                nc.tensor.matmul(
                                    sc[:, slot, :], kst,
                                    qv[:, 2 * cp:2 * cp + 2, qs],
                                    start=(cp == 0 and i == 0),
                                    stop=(cp == 1 and i == len(qvs) - 1),
                                    perf_mode=mybir.MatmulPerfMode.DoubleRow,
                                )
                    nc.scalar.activation(
                        pT_cache[:, mb, pr].rearrange("p a b -> p (a b)"),
                        sc[:].rearrange("p a b -> p (a b)"),
                        mybir.ActivationFunctionType.Exp,
                        bias=zeros_col[:], scale=SCALE,
                    )

                # sweep: scores both mb per pair; attention mb0 inline
                for pr in range(NPR):
                    scores(pr, 0)
                    scores(pr, 1)
                    if pr > 0:
                        attn(hs0, 0, pr - 1, pr == 1, False)
                attn(hs0, 0, NPR - 1, False, True)

                sc_ctx.close()

                with tc.tile_pool(name="ps_d0", bufs=1, space="PSUM") as ps_d0:
                    den0 = ps_d0.tile([P, D], F32)
                    den_pass(den0, 0)
                    norm_out(hs0, den0, 0, scr, rcpw[:, 0, :])

            # mb1 attention phase (banks vacated by score tiles).
            # den1 runs FIRST so its transposes+reciprocal (DVE) overlap
            # attn1's PE work; the tail is then just 4 ACT scales + DMA.
            with (
                tc.tile_pool(name="ps_hs1", bufs=1, space="PSUM") as ps_hs1,
                tc.tile_pool(name="ps_d1", bufs=1, space="PSUM") as ps_d1,
            ):
                hs1 = ps_hs1.tile([P, 4, D], F32)
                den1 = ps_d1.tile([P, D], F32)
                for pr in range(NPR):
                    nc.tensor.matmul(
                        den1[:], ones8[:], pT_cache[:, 1, pr],
                        start=(pr == 0), stop=(pr == NPR - 1),
                        perf_mode=mybir.MatmulPerfMode.DoubleRow,
                    )
                rc1 = rcpw[:, 1, :]
                for mt in range(4):
                    for x in range(4):
                        nc.vector.transpose(
                            scr[32 * x:32 * x + 32, mt * 32:(mt + 1) * 32],
                            den1[32 * x:32 * x + 32,
                                 mt * P + 32 * x:mt * P + 32 * x + 32])
                    nc.vector.reciprocal(rc1[:, mt:mt + 1],
                                         scr[:, mt * 32:mt * 32 + 1])
                for pr in range(NPR):
                    attn(hs1, 1, pr, pr == 0, pr == NPR - 1)
                h_dv = h_d.ap().rearrange("(t p) d -> p t d", p=P)
                for mt in range(4):
                    ot = outp.tile([P, D], F32, name="ot", tag="ot")
                    nc.vector.scalar_tensor_tensor(
                        ot[:], hs1[:, mt, :], rc1[:, mt:mt + 1], bv_rep[:],
                        op0=mybir.AluOpType.mult,
                        op1=mybir.AluOpType.add)
                    nc.sync.dma_start(h_dv[:, 4 + mt, :], ot[:])
            outp_ctx.close()

    nc.compile()
    return nc


_cache = {}


def kernel(z, Wq, bq, Wk, bk, Wv, bv):
    if "nc" not in _cache:
        _cache["nc"] = _build()
    nc = _cache["nc"]

    bf16 = ml_dtypes.bfloat16
    f8 = ml_dtypes.float8_e4m3
    z, Wq, bq, Wk, bk, Wv, bv = (np.asarray(t) for t in
                                 (z, Wq, bq, Wk, bk, Wv, bv))
    z = np.ascontiguousarray(z, dtype=np.float32)
    zT = np.ascontiguousarray(z.T).astype(bf16)
    base = {
        "WqT": np.ascontiguousarray(Wq.T).astype(bf16),
        "WkT": np.ascontiguousarray(Wk.T).astype(bf16),
        "WvT": np.ascontiguousarray(Wv.T).astype(bf16),
        "bq": np.ascontiguousarray(bq, dtype=np.float32),
        "bv_rep": np.broadcast_to(
            np.asarray(bv, dtype=np.float32)[None, :], (P, D)).copy(),
        "ones8": np.ones((P, 2, P), dtype=f8),
    }
    in_maps = []
    for i in range(N_CORES):
        m = dict(base)
        cols = [(i * M_LOC + t) % N_TOK for t in range(2 * M_LOC)]
        m["zT_loc"] = np.ascontiguousarray(zT[:, cols])
        rem = [((i + 2 + j) % N_CORES) * KV_SL for j in range(N_CORES - 2)]
        m["offs"] = np.array([rem + [r + KT_SL for r in rem]], dtype=np.int32)
        in_maps.append(m)

    _cache["in_maps"] = in_maps
    res = run_bass_kernel_spmd(nc, in_maps, core_ids=list(range(N_CORES)))
    _cache["last_result"] = res
    return np.concatenate(
        [res.results[i]["h_out"] for i in range(N_CORES)], axis=0)
